# revision 9
# baseline (speedup 1.0000x reference)
"""AnomalyAttention (causal multi-head attention) on 8 TRN2 NeuronCores.

Problem: B=2, C=4, H=8, L=1024, E=64  ->  64 independent heads.
Sharding: 8 heads per core (data parallel over B*C*H), no collectives.

Per-core pipeline (heads processed in pairs; head 2p uses PE row group 0:64,
head 2p+1 uses 64:128 so their QK^T matmuls run concurrently in the array):
  S^T[k, q] = sum_e K[k, e] Q[q, e]        (TensorE; lhsT = K^T chunk, rhs = Q^T)
  P^T = exp(0.125 * S^T)  (ScalarE, bf16)  causal: upper blocks skipped,
                                           diagonal blocks * tri 0/1 (VectorE)
  [O | r][q, :] = sum_k P^T[k, q] * [V | 1][k, :]   (TensorE, PSUM accumulate)
  out[q, e] = O[q, e] / r[q]               (VectorE reciprocal + broadcast mult)

Inputs are pre-transposed / bf16-cast / padded on the host as part of sharding.
Output DRAM layout [h, q%128, (q//128)*64 + e] for contiguous per-partition DMA.
"""

import os
import numpy as np
from ml_dtypes import bfloat16

import concourse.bass as bass
import concourse.tile as tile
from concourse import bacc, mybir
from concourse.bass_utils import run_bass_kernel_spmd

B, C, H, L, E = 2, 4, 8, 1024, 64
N_CORES = 8
HEADS = B * C * H              # 64
HPC = HEADS // N_CORES         # 8 heads per core
NPAIR = HPC // 2               # 4 head pairs per core
NCHUNK = L // 128              # 8 k-chunks of 128
SCALE = 1.0 / 8.0

LAST_RESULTS = None


def _bank_splits(c0, c1):
    """Split tile-column range [c0, c1) at 512-aligned boundaries."""
    cuts = [c0]
    nb = (c0 // 512 + 1) * 512
    while nb < c1:
        cuts.append(nb)
        nb += 512
    cuts.append(c1)
    return list(zip(cuts[:-1], cuts[1:]))


class _Core:
    """Holds tiles + emission helpers for one SPMD program."""

    def __init__(self, nc, pools, qT_t, kT_t, vo_t, tri_t, msk_t, out, causal):
        self.nc = nc
        self.pools = pools
        self.qT_t, self.kT_t, self.vo_t = qT_t, kT_t, vo_t
        self.tri_t, self.msk_t = tri_t, msk_t
        self.out = out
        self.causal = causal
        self.pmaps = {}

    def s_tile(self, p, t):
        """Emit S^T + exp + mask for pair-chunk tile t of head pair p.
        Heads 2p / 2p+1 are interleaved on PE row groups 0:64 / 64:128."""
        nc = self.nc
        bf = mybir.dt.bfloat16
        f32 = mybir.dt.float32
        psumS, pTpool = self.pools["psumS"], self.pools["pT"]
        a, b2 = t, NCHUNK - 1 - t
        if self.causal:
            wa, wb = L - 128 * a, L - 128 * b2
            segs = [(a, 0, wa, 128 * a), (b2, wa, wa + wb, 128 * b2)]
            width = wa + wb  # 1152
        else:
            # non-causal: full-width chunks, one chunk per psum tile
            segs = None

        for h_loc, rows in ((0, slice(0, 64)), (1, slice(64, 128))):
            h = 2 * p + h_loc
            self.pmaps.setdefault(h, [None] * NCHUNK)

        if self.causal:
            ps = [psumS.tile([128, 1152], f32, tag="psS", name="psS") for _ in range(2)]
            # chunk-level ping-pong across the two row groups so LDWEIGHTS
            # of one head overlaps the other head's streaming matmul
            for (ci, c0, c1, q0) in segs:
                for h_loc, rows in ((0, slice(0, 64)), (1, slice(64, 128))):
                    hp = p
                    for s0, s1 in _bank_splits(c0, c1):
                        qs = q0 + (s0 - c0)
                        nc.tensor.matmul(
                            ps[h_loc][:, s0:s1],
                            lhsT=self.kT_t[hp][rows, 128 * ci:128 * ci + 128],
                            rhs=self.qT_t[hp][rows, qs:qs + (s1 - s0)],
                            start=True, stop=True,
                        )
            for h_loc in (0, 1):
                h = 2 * p + h_loc
                pt = pTpool.tile([128, 1152], bf, tag="pt", name="pt")
                nc.scalar.activation(
                    pt[:, 0:width], ps[h_loc][:, 0:width],
                    mybir.ActivationFunctionType.Exp, scale=SCALE,
                )
                # both diagonal blocks in one 2-region strided op
                ptap = pt[:, :]
                src = bass.AP(tensor=ptap.tensor, offset=ptap.offset,
                              ap=[ptap.ap[0], [wa, 2], [1, 128]])
                tap = self.tri_t[:, :]
                trib = bass.AP(tensor=tap.tensor, offset=tap.offset,
                               ap=[tap.ap[0], [0, 2], [1, 128]])
                nc.vector.tensor_mul(src, src, trib)
                self.pmaps[h][a] = (pt, 0)
                self.pmaps[h][b2] = (pt, wa)
        else:
            for (ci,) in ((a,), (b2,)):
                ps = [psumS.tile([128, 1152], f32, tag="psS", name="psS") for _ in range(2)]
                for s0 in range(0, L, 512):
                    for h_loc, rows in ((0, slice(0, 64)), (1, slice(64, 128))):
                        nc.tensor.matmul(
                            ps[h_loc][:, s0:s0 + 512],
                            lhsT=self.kT_t[p][rows, 128 * ci:128 * ci + 128],
                            rhs=self.qT_t[p][rows, s0:s0 + 512],
                            start=True, stop=True,
                        )
                for h_loc in (0, 1):
                    h = 2 * p + h_loc
                    pt = pTpool.tile([128, 1152], bf, tag="pt", name="pt")
                    nc.scalar.activation(
                        pt[:, 0:L], ps[h_loc][:, 0:L],
                        mybir.ActivationFunctionType.Exp, scale=SCALE,
                    )
                    nc.vector.tensor_mul(pt[:, 0:L], pt[:, 0:L],
                                         self.msk_t[ci])
                    self.pmaps[h][ci] = (pt, 0)

    def av_unit(self, h, jp, obuf):
        """Emit AV + normalize for q-blocks 2jp, 2jp+1 of head h."""
        nc = self.nc
        f32 = mybir.dt.float32
        psumO, rinvp = self.pools["psumO"], self.pools["rinvp"]
        pmap = self.pmaps[h]
        po = psumO.tile([128, 130], f32, tag="psO", name="psO")
        for j, coff in ((2 * jp, 0), (2 * jp + 1, 65)):
            ks = list(range(j + 1)) if self.causal else list(range(NCHUNK))
            for idx, i in enumerate(ks):
                pt, off = pmap[i]
                cs = off + 128 * (j - i) if self.causal else 128 * j
                nc.tensor.matmul(
                    po[:, coff:coff + 65],
                    lhsT=pt[:, cs:cs + 128],
                    rhs=self.vo_t[h][:, 65 * i:65 * i + 65],
                    start=(idx == 0), stop=(idx == len(ks) - 1),
                )
        # reciprocal of the two denominators (cols 64 and 129)
        rinv2 = rinvp.tile([128, 2], f32, tag="rinv", name="rinv")
        poap = po[:, :]
        rsrc = bass.AP(tensor=poap.tensor, offset=poap.offset + 64,
                       ap=[poap.ap[0], [65, 2]])
        nc.vector.reciprocal(rinv2, rsrc)
        # obuf[:, 128*jp : 128*jp+128] = po{cols 0:64, 65:129} * rinv2 bcast
        o_in = bass.AP(tensor=poap.tensor, offset=poap.offset,
                       ap=[poap.ap[0], [65, 2], [1, 64]])
        rap = rinv2[:, :]
        r_in = bass.AP(tensor=rap.tensor, offset=rap.offset,
                       ap=[rap.ap[0], [1, 2], [0, 64]])
        oap = obuf[:, :]
        o_out = bass.AP(tensor=oap.tensor, offset=oap.offset + 128 * jp,
                        ap=[oap.ap[0], [64, 2], [1, 64]])
        nc.vector.tensor_mul(o_out, o_in, r_in)


def _build(causal: bool):
    nc = bacc.Bacc("TRN2", target_bir_lowering=False, debug=False,
                   num_devices=N_CORES)
    bf = mybir.dt.bfloat16
    f32 = mybir.dt.float32

    qT = nc.dram_tensor("qT", [NPAIR, 128, L], bf, kind="ExternalInput").ap()
    kT = nc.dram_tensor("kT", [NPAIR, 128, L], bf, kind="ExternalInput").ap()
    vo = nc.dram_tensor("vo", [HPC, 128, NCHUNK * 65], bf, kind="ExternalInput").ap()
    tri = nc.dram_tensor("tri", [128, 128], bf, kind="ExternalInput").ap()
    if not causal:
        msk = nc.dram_tensor("msk", [NCHUNK, 128, L], bf, kind="ExternalInput").ap()
    out = nc.dram_tensor("out", [HPC, 128, 512], f32, kind="ExternalOutput").ap()

    with tile.TileContext(nc) as tc:
        with (
            tc.tile_pool(name="consts", bufs=1) as consts,
            tc.tile_pool(name="pT", bufs=16) as pTpool,
            tc.tile_pool(name="psumS", bufs=2, space="PSUM") as psumS,
            tc.tile_pool(name="psumO", bufs=2, space="PSUM") as psumO,
            tc.tile_pool(name="outsb", bufs=2) as outsb,
            tc.tile_pool(name="rinvp", bufs=4) as rinvp,
        ):
            pools = dict(psumS=psumS, psumO=psumO, pT=pTpool, rinvp=rinvp)
            # input DMAs spread over four engine queues so the first head
            # pair's tensors land ASAP and transfers run in parallel
            qT_t = [consts.tile([128, L], bf, tag=f"qT{p}", name=f"qTt{p}")
                    for p in range(NPAIR)]
            kT_t = [consts.tile([128, L], bf, tag=f"kT{p}", name=f"kTt{p}")
                    for p in range(NPAIR)]
            vo_t = [consts.tile([128, NCHUNK * 65], bf, tag=f"vo{h}",
                                name=f"vot{h}")
                    for h in range(HPC)]
            tri_t = consts.tile([128, 128], bf, tag="tri")
            # single HWDGE queue, need-ordered: serial descriptor generation
            # staggers the transfers so the first head pair's tensors get
            # full HBM bandwidth instead of fair-sharing with later loads
            nc.sync.dma_start(out=kT_t[0], in_=kT[0])
            nc.sync.dma_start(out=qT_t[0], in_=qT[0])
            nc.scalar.dma_start(out=tri_t, in_=tri)
            nc.sync.dma_start(out=vo_t[0], in_=vo[0])
            nc.sync.dma_start(out=vo_t[1], in_=vo[1])
            order = [("k", 1), ("q", 1), ("v", 2), ("v", 3), ("k", 2),
                     ("q", 2), ("v", 4), ("v", 5), ("k", 3), ("q", 3),
                     ("v", 6), ("v", 7)]
            for kind, i in order:
                t = {"k": kT_t, "q": qT_t, "v": vo_t}[kind][i]
                s = {"k": kT, "q": qT, "v": vo}[kind][i]
                nc.sync.dma_start(out=t, in_=s)
            msk_t = []
            if not causal:
                for c in range(NCHUNK):
                    t = consts.tile([128, L], bf, tag=f"msk{c}", name=f"mskt{c}")
                    nc.gpsimd.dma_start(out=t, in_=msk[c])
                    msk_t.append(t)

            # warm the ACT exp table while DMAs run
            warm = consts.tile([128, 8], f32, tag="warm")
            nc.vector.memset(warm, 0.0)
            warm2 = consts.tile([128, 8], f32, tag="warm2")
            nc.scalar.activation(warm2, warm, mybir.ActivationFunctionType.Exp)

            core = _Core(nc, pools, qT_t, kT_t, vo_t, tri_t, msk_t, out, causal)

            # software pipeline: S tiles of pair p+1 interleaved with AV units
            # of pair p, so PE always has independent matmul work while ACT
            # drains the exp backlog.
            for t in range(4):
                core.s_tile(0, t)
            obufs = {}
            for p in range(NPAIR):
                for h in (2 * p, 2 * p + 1):
                    obufs[h] = outsb.tile([128, 512], f32, tag=f"ob{h % 4}", name=f"ob{h}")
                units = [(h, jp) for jp in range(4) for h in (2 * p, 2 * p + 1)]
                last = p + 1 == NPAIR
                ui = 0
                if not last:
                    for t in range(4):
                        core.s_tile(p + 1, t)
                        for _ in range(2):
                            core.av_unit(*units[ui], obufs[units[ui][0]])
                            ui += 1
                while ui < len(units):
                    h, jp = units[ui]
                    core.av_unit(h, jp, obufs[h])
                    if last:
                        # stream each finished 128-col slab out immediately,
                        # alternating queues so descriptor gen doesn't serialize
                        eng = (nc.sync, nc.gpsimd, nc.scalar)[ui % 3]
                        oap = obufs[h][:, 128 * jp:128 * jp + 128]
                        eng.dma_start(
                            out=out[h][:, 128 * jp:128 * jp + 128], in_=oap)
                    ui += 1
                for h in (2 * p, 2 * p + 1):
                    ob = obufs.pop(h)
                    if not last:
                        nc.sync.dma_start(out=out[h], in_=ob)
    nc.compile()
    return nc


_CACHE = {}


def _get_nc(causal: bool):
    if causal not in _CACHE:
        _CACHE[causal] = _build(causal)
    return _CACHE[causal]


def kernel(queries, keys, values, attn_mask):
    global LAST_RESULTS
    q = np.asarray(queries).reshape(HEADS, L, E)
    k = np.asarray(keys).reshape(HEADS, L, E)
    v = np.asarray(values).reshape(HEADS, L, E)
    mask = np.asarray(attn_mask).reshape(L, L)
    causal = bool(np.array_equal(mask, np.triu(np.ones((L, L), bool), k=1)))

    nc = _get_nc(causal)

    tri = np.triu(np.ones((128, 128), np.float32), k=0).astype(bfloat16)
    if not causal:
        m01 = np.where(mask, 0.0, 1.0).astype(np.float32)  # [k, q] keep=1
        msk = m01.reshape(NCHUNK, 128, L).astype(bfloat16)

    in_maps = []
    for c in range(N_CORES):
        hs = slice(c * HPC, (c + 1) * HPC)
        qTm = np.ascontiguousarray(
            q[hs].transpose(0, 2, 1)).astype(bfloat16).reshape(NPAIR, 128, L)
        kTm = np.ascontiguousarray(
            k[hs].transpose(0, 2, 1)).astype(bfloat16).reshape(NPAIR, 128, L)
        vh = v[hs].astype(np.float32)
        vcat = np.concatenate(
            [vh, np.ones((HPC, L, 1), np.float32)], axis=2)  # [8, L, 65]
        vom = np.ascontiguousarray(
            vcat.reshape(HPC, NCHUNK, 128, 65).transpose(0, 2, 1, 3)
        ).astype(bfloat16).reshape(HPC, 128, NCHUNK * 65)
        im = {"qT": qTm, "kT": kTm, "vo": vom, "tri": tri}
        if not causal:
            im["msk"] = msk
        in_maps.append(im)

    trace = bool(os.environ.get("BASS_ATTN_TRACE"))
    res = run_bass_kernel_spmd(nc, in_maps, core_ids=list(range(N_CORES)),
                               trace=trace)
    LAST_RESULTS = res
    # out[c]: [HPC, 128, 512] = [h, p, j*64+e]; q = 128*j + p
    outs = np.stack([res.results[c]["out"] for c in range(N_CORES)])
    outs = outs.reshape(N_CORES, HPC, 128, NCHUNK, E).transpose(0, 1, 3, 2, 4)
    return np.ascontiguousarray(
        outs.reshape(B, C, H, L, E)).astype(np.float32)
